# revision 29
# baseline (speedup 1.0000x reference)
"""Ergodicity loss kernel for Trainium2 (8 NeuronCores, batch-sharded SPMD).

Math: loss = mean((c - coeffs)^2) + REG*sum(u^2)/(2*N*T*B)
      c[b,i,j] = sum_{t,n} cos(i*pi*x0)*cos(j*pi*x1) / (norm[i,j]*N*T)

Device computes, per core (4 of 32 batches; batch-sharded so no collective):
  - 16 "feature" tensors per spatial dim: fixed linear mixes of cos(k*pi*x_d)
    built from one ACT Sin (k=1), ACT Square ops (affine folded into the
    activation scale/bias; evens f2,f4,f8,f10,f12) and DVE ops (one STT for
    the clean -c3/4, plain tensor_muls for the rest).  Conditioning of the
    mixing matrix is checked row-normalized (device errors are relative).
  - Feature storage is a block layout: column = blk*256 + k*16 + nl*2 + d
    with blk = (tc*4+b)*8+oc, so matmul operands are single-free-dim
    [[2,128]] APs (the walrus verifier requires that) while feature ops see
    [[256,n],[1,16]] APs (16-elem unit-stride runs for DVE 2x_1P).
  - C'[b, k0*8+n, k1*8+n] via accumulating bf16 matmuls; off-diagonal
    n-cells are junk, dropped on the host.
  - sum(u^2) on the PE: fp16 Gram-block self-matmuls into one psum tile
    (host sums the diagonal).  Inputs arrive as fp16 (host-cast), halving
    HBM traffic; x comes as two 2 KB-per-partition-row chunks (1 KB rows
    halve effective DMA bandwidth).
  - A junk-matmul stream at kernel start keeps the PE HAM activity monitor
    busy so the PE clock is at 2.4 GHz for the u-gram / first bursts.

The Sin activation LUT is only accurate for |arg| <= ~pi/2 (measured via
the probe), hence exactly one Sin per x element and product chains for all
higher harmonics.

Host recovers the true cos-basis C by inverting the feature-mixing matrix A
(replayed symbolically in a cos-harmonic algebra), then finishes in float64.

Toolchain notes: this walrus build enforces a 1-sync-wait budget on most
instruction templates.  Structural consequences: engine-internal program
order is pinned with sync-free dep edges (Tile's scheduler otherwise
reorders and breaks the opener trick), an "opener" matmul per slab
pre-observes the ACT semaphore on the PE so real matmuls carry only the
one DVE wait their template allows, activation bias constants ride in the
"pr" input DMA (observed once by the first ACT op) instead of pre-barrier
gpsimd memsets, and the kernel-tail barrier is split into per-proc drains.
"""

import sys

sys.path.insert(0, "/opt/trn_rl_repo")

import numpy as np

import concourse.bass as bass
import concourse.mybir as mybir
from concourse import bass_utils
from concourse.tile import TileContext
from concourse.tile_rust import add_dep_helper
from concourse.vector_clock import ScopedClock, VectorClock

_orig_drain_and_barrier = TileContext._drain_and_barrier


def _split_drain_and_barrier(self, tick_clock, wait_clock):
    gc = tick_clock.global_clock
    ticks = list(gc)
    procs = [i for i, t in enumerate(ticks) if t > 0]
    for p in procs:
        vec = [0] * len(ticks)
        vec[p] = ticks[p]
        d = self.nc.sync.drain()
        wait_clock.add_sem_waits(d.ins, ScopedClock({None: VectorClock(vec)}))
    self.nc.all_engine_barrier(sem_only=True)
    popped = self.nc._tile_sem_poison_stack.pop()
    assert popped is self._sem_poison
    self.nc.clear_and_free_semaphores(list(self.sems.allocated().values()))
    self.nc.all_engine_barrier(sem_only=True)


TileContext._drain_and_barrier = _split_drain_and_barrier

# Problem constants (hardcoded per spec).
K_MAX = 16
N_AGENTS = 64
T = 512
B = 32
D = 2
REG = 1e-3
N_CORES = 8
BPC = B // N_CORES  # batches per core = 4

PI = float(np.pi)

F32 = mybir.dt.float32
F16 = mybir.dt.float16
BF16 = mybir.dt.bfloat16

# Per-core geometry: x shard [T=512, BPC=4, N=64, D=2] is host-permuted to
# [128, 2048] fp16 with partition p = t % 128 and column
# tc*512 + b*128 + n*2 + d  (tc = t // 128).
TC = 4
COLS = TC * BPC * N_AGENTS * D  # 2048
NBLK = 128  # feature blocks (tc, b, oc)
NPROBE = 16
NPR = NPROBE + 3  # probe cols + {-pi/2, -1.0, 0.0} activation-bias consts




# ---------------------------------------------------------------------------
# Symbolic harmonic algebra.
# ---------------------------------------------------------------------------
class Harm:
    __slots__ = ("c",)

    def __init__(self, c):
        self.c = np.asarray(c, dtype=np.float64)

    @staticmethod
    def const(v):
        c = np.zeros(K_MAX)
        c[0] = v
        return Harm(c)

    @staticmethod
    def basis(k, v=1.0):
        c = np.zeros(K_MAX)
        c[k] = v
        return Harm(c)

    def affine(self, scale, bias):
        c = self.c * scale
        c[0] += bias
        return Harm(c)

    def mul(self, other):
        out = np.zeros(K_MAX)
        for a in range(K_MAX):
            if self.c[a] == 0.0:
                continue
            for b in range(K_MAX):
                if other.c[b] == 0.0:
                    continue
                v = self.c[a] * other.c[b]
                s, d = a + b, abs(a - b)
                assert s < K_MAX or v == 0.0, f"harmonic overflow {a}+{b}"
                out[s] += 0.5 * v
                out[d] += 0.5 * v
        return Harm(out)

    def square(self, scale=1.0, bias=0.0):
        z = self.affine(scale, bias)
        return z.mul(z)

    def stt(self, s, other):  # (self - s) * other
        return self.affine(1.0, -s).mul(other)


def _feature_mixing_matrix():
    """Replay the device feature pipeline symbolically -> A[16,16].
    Must mirror the ops in _body exactly."""
    f = [None] * K_MAX
    f[0] = Harm.const(1.0)
    f[1] = Harm.basis(1, -1.0)      # ACT Sin(pi*x - pi/2) = -cos(pi*x)
    f[2] = f[1].square()             # ACT
    f[4] = f[2].square(2.0, -1.0)    # ACT
    f[8] = f[4].square(2.0, -1.0)    # ACT
    f[3] = f[2].stt(0.75, f[1])      # DVE STT -> -c3/4
    f[6] = f[3].mul(f[3])            # DVE TT -> (c6+1)/32
    f[5] = f[4].mul(f[1])            # DVE TT
    f[7] = f[6].mul(f[1])            # DVE TT
    f[10] = f[5].square(4.0, 0.0)    # ACT
    f[12] = f[6].square(32.0, -1.0)  # ACT -> (c12+1)/2
    f[9] = f[8].mul(f[1])            # DVE TT
    f[14] = f[7].mul(f[7])           # DVE TT
    f[11] = f[10].mul(f[1])          # DVE leaf
    f[13] = f[12].mul(f[1])          # DVE leaf
    f[15] = f[14].mul(f[1])          # DVE leaf
    return np.stack([x.c for x in f])


_A = _feature_mixing_matrix()
_AINV = np.linalg.inv(_A)
_ROWCOND = np.linalg.cond(_A / np.linalg.norm(_A, axis=1, keepdims=True))
assert _ROWCOND < 1e3, _ROWCOND


def _np_constants():
    ks = np.arange(K_MAX, dtype=np.float64)
    vs = []
    for _ in range(D):
        with np.errstate(divide="ignore", invalid="ignore"):
            ki = ks * np.pi
            nz = (np.exp(1j * ki) - 1.0) / (1j * ki)
        integral = np.where(ks == 0, 1.0 + 0j, nz)
        vs.append(integral)
    cd = np.real(vs[0][:, None] * vs[1][None, :]).astype(np.float64)
    norm_last = np.where(ks == 0, 1.0, np.sqrt(0.5))
    norm = np.broadcast_to(norm_last[None, :], (K_MAX, K_MAX)).copy()
    return cd / norm, norm


_COEFFS, _NORM = _np_constants()


# ---------------------------------------------------------------------------
# Device program
# ---------------------------------------------------------------------------
def _body(nc, tc, xu_in, pr_in, out_dram):
    Sq = mybir.ActivationFunctionType.Square
    Sin = mybir.ActivationFunctionType.Sin
    sub = mybir.AluOpType.subtract
    mult = mybir.AluOpType.mult

    with (
        tc.tile_pool(name="io", bufs=1) as io_pool,
        tc.tile_pool(name="feat", bufs=1) as feat_pool,
        tc.tile_pool(name="work", bufs=1) as work_pool,
        tc.tile_pool(name="psum", bufs=1, space="PSUM") as psum_pool,
    ):
        xu = io_pool.tile([128, 2 * COLS], F16, tag="xu")
        pr = io_pool.tile([128, NPR], F32, tag="pr")
        # Activation bias consts ride in the pr DMA; the probe Sin (first
        # ACT op) observes that queue once for the whole ACT stream.
        nc.const_aps.aps[(F32, -PI / 2)] = pr[:, NPROBE : NPROBE + 1]
        nc.const_aps.aps[(F32, -1.0)] = pr[:, NPROBE + 1 : NPROBE + 2]
        nc.const_aps.aps[(F32, 0.0)] = pr[:, NPROBE + 2 : NPROBE + 3]

        # DMA kicks (each dma_start = one HW queue, kick instructions
        # serialize ~0.65us each on SP): x first (it gates the ACT chain),
        # pr (bias consts, needed by the probe at ~8us) last.
        HC = COLS // 2
        nc.sync.dma_start(out=xu[:, 0:HC], in_=xu_in[:, 0:HC])
        nc.sync.dma_start(out=xu[:, HC:COLS], in_=xu_in[:, HC:COLS])
        nc.sync.dma_start(out=xu[:, COLS:], in_=xu_in[:, COLS:])
        nc.sync.dma_start(out=pr[:], in_=pr_in[:])
        uraw = xu[:, COLS : 2 * COLS]

        # Feature storage block layout (see module docstring).
        FA = feat_pool.tile([128, K_MAX * COLS], BF16, tag="FA")
        FAke = FA[:].rearrange(
            "p (blk k e) -> p k blk e", blk=NBLK, k=K_MAX, e=16
        )
        FAm = FA[:].rearrange(
            "p (blk kn d) -> p blk d kn", blk=NBLK, kn=128, d=D
        )

        def F(k, b0=0, b1=NBLK):
            return FAke[:, k, b0:b1]

        warm = work_pool.tile([128, 128], BF16, tag="warm")
        # csb layout: [probe 16][psu 128][ps0..ps3 512] so each output DMA
        # covers one contiguous span and the b3 tail DMA is small.
        csb = work_pool.tile([128, NPROBE + 128 + BPC * 128], F32, tag="csb")

        pstiles = [
            psum_pool.tile([128, 128], F32, tag=f"ps{b}", name=f"ps{b}")
            for b in range(BPC)
        ]
        psu = psum_pool.tile([128, 128], F32, tag="psu")
        pjunk = psum_pool.tile([128, 128], F32, tag="pjunk")

        # Pin engine-internal program order (Tile reorders otherwise).
        _last = {}

        def _pin(key, bi):
            if key in _last:
                add_dep_helper(bi.ins, _last[key].ins, sync=False,
                               reason=f"{key} order pin")
            _last[key] = bi
            return bi

        # GPSIMD: f0 = ones first, warm second; the first junk matmul's
        # single gpsimd wait covers both.
        _pin("gp", nc.gpsimd.memset(F(0), 1.0))
        _pin("gp", nc.gpsimd.memset(warm[:], 1.0))

        HB = NBLK // 2
        SL = {0: (0, HB), 1: (HB, NBLK)}

        def act(out, in_, func, **kw):
            return _pin("act", nc.scalar.activation(out, in_, func, **kw))

        def stt(k_out, k_in, s, k_mul, sl):
            b0, b1 = sl
            return _pin("dve", nc.vector.scalar_tensor_tensor(
                F(k_out, b0, b1), F(k_in, b0, b1), s, F(k_mul, b0, b1),
                sub, mult,
            ))

        def tt(k_out, k_a, k_b, sl):
            b0, b1 = sl
            return _pin("dve", nc.vector.tensor_mul(
                out=F(k_out, b0, b1), in0=F(k_a, b0, b1), in1=F(k_b, b0, b1)
            ))

        def sq(k_out, k_in, sl, scale=1.0, bias=0.0):
            b0, b1 = sl
            return act(F(k_out, b0, b1), F(k_in, b0, b1), Sq,
                       scale=scale, bias=bias)

        # --- ACT + DVE streams ---
        # Issue order MUST be topological (producer before consumer): Tile
        # tracks dataflow by issue order; a consumer issued before its
        # producer reads "uninitialized" bytes and the real write becomes a
        # reversed WAR anti-dep.  Per-engine execution order = issue order
        # (pinned above).  ACT order is by downstream urgency: the leaf
        # muls (f11, f13) that gate the matmul bursts need f10/f12 as early
        # as their DVE inputs allow.
        act(csb[:, :NPROBE], pr[:, :NPROBE], Sin, scale=1.0)         # probe
        act(F(1, 0, HB), xu[:, 0:HC], Sin, scale=PI, bias=-PI / 2)   # sin s0
        act(F(1, HB, NBLK), xu[:, HC:COLS], Sin, scale=PI, bias=-PI / 2)
        sq(2, 1, SL[0])
        sq(2, 1, SL[1])
        stt(3, 2, 0.75, 1, SL[0])
        stt(3, 2, 0.75, 1, SL[1])
        sq(4, 2, SL[0], 2.0, -1.0)
        sq(4, 2, SL[1], 2.0, -1.0)
        tt(6, 3, 3, SL[0])
        tt(6, 3, 3, SL[1])
        sq(8, 4, SL[0], 2.0, -1.0)
        tt(5, 4, 1, SL[0])
        tt(7, 6, 1, SL[0])
        tt(14, 7, 7, SL[0])
        sq(12, 6, SL[0], 32.0, -1.0)
        sq(10, 5, SL[0], 4.0, 0.0)
        sq(8, 4, SL[1], 2.0, -1.0)
        tt(5, 4, 1, SL[1])
        tt(7, 6, 1, SL[1])
        tt(14, 7, 7, SL[1])
        sq(12, 6, SL[1], 32.0, -1.0)
        sq(10, 5, SL[1], 4.0, 0.0)
        tt(9, 8, 1, SL[0])
        # slab-0 leaves, per tc so the tc0 burst starts early
        for tcc in (0, 1):
            hs = (tcc * 32, (tcc + 1) * 32)
            tt(15, 14, 1, hs)
            tt(11, 10, 1, hs)
            tt(13, 12, 1, hs)
        tt(9, 8, 1, SL[1])
        for tcc in (2, 3):
            hs = (tcc * 32, (tcc + 1) * 32)
            tt(15, 14, 1, hs)
            tt(11, 10, 1, hs)
            tt(13, 12, 1, hs)

        # --- PE stream ---
        last_mm = None

        def junk(n):
            # 1-column matmuls: keep the HAM activity monitor busy with
            # ~no SBUF traffic (full-width junk matmuls stream 64 KB each
            # and visibly steal SBUF bandwidth from ACT/DVE).
            nonlocal last_mm
            for _ in range(n):
                mm = nc.tensor.matmul(
                    pjunk[0:1, 100:101], warm[:, 0:1], warm[:, 0:1],
                    start=True, stop=True, skip_group_check=True,
                )
                if last_mm is not None:
                    add_dep_helper(mm.ins, last_mm.ins, sync=False,
                                   reason="junk chain")
                last_mm = mm

        for c in range(16):
            ub = uraw[:, c * 128 : (c + 1) * 128]
            mm = nc.tensor.matmul(
                psu[:], ub, ub, start=(c == 0), stop=(c == 15),
                skip_group_check=True,
            )
            if last_mm is not None:
                add_dep_helper(mm.ins, last_mm.ins, sync=False, reason="ugram")
            last_mm = mm
        # Warm the PE clock for the real bursts: the HAM un-throttles only
        # after ~3.4us of SUSTAINED busy, so tiny keep-alive matmuls are
        # not enough.  Run a ~3.5us stream of 32-col junk matmuls gated on
        # f8 of slab 1 (~3.5us before the first burst); 32-col operands
        # keep the SBUF read traffic low enough not to rob ACT/DVE
        # (full-width junk measurably slowed every ACT op by ~20%).
        stub = FAke[:, 8, NBLK - 1 : NBLK, 15:16]
        gate = nc.tensor.matmul(
            pjunk[0:1, 110:111], stub, stub,
            start=True, stop=True, skip_group_check=True,
        )
        add_dep_helper(gate.ins, last_mm.ins, sync=False, reason="warm gate")
        last_mm = gate
        for _ in range(70):
            mm = nc.tensor.matmul(
                pjunk[0:32, 64:96], warm[:, 0:32], warm[:, 0:32],
                start=True, stop=True, skip_group_check=True,
            )
            add_dep_helper(mm.ins, last_mm.ins, sync=False, reason="warm32")
            last_mm = mm

        # Real bursts: per slab an opener matmul reads the slab's LAST
        # ACT-written feature (f10, per the pinned ACT order) so real
        # Ldweights carry only their single DVE wait.
        for si in (0, 1):
            b0, b1 = SL[si]
            fstub = FAke[:, 10, b1 - 1 : b1, 15:16]
            op = nc.tensor.matmul(
                pjunk[0:1, 120 + si : 121 + si], fstub, fstub,
                start=True, stop=True, skip_group_check=True,
            )
            add_dep_helper(op.ins, last_mm.ins, sync=False,
                           reason="opener after prev MMs")
            opener = op
            last_mm = op
            for tcc in (2 * si, 2 * si + 1):
                for b in range(BPC):
                    ps = pstiles[b]
                    for oc in range(8):
                        blk = (tcc * BPC + b) * 8 + oc
                        mm = nc.tensor.matmul(
                            ps[:], FAm[:, blk, 0], FAm[:, blk, 1],
                            start=(tcc == 0 and oc == 0),
                            stop=(tcc == TC - 1 and oc == 7),
                            skip_group_check=True,
                        )
                        add_dep_helper(mm.ins, opener.ins, sync=False,
                                       reason="PE wait-slot opener")
                        last_mm = mm

        # PSUM -> SBUF on ACT; one output DMA per contiguous span so the
        # output transfer overlaps the burst tail instead of serializing
        # 328 KB after the last copy.  probe+psu ride with b0's DMA to stay
        # within the 8 HW queues (4 in + 4 out).
        _pin("act", nc.scalar.copy(
            out=csb[:, NPROBE : NPROBE + 128], in_=psu[:]))
        PS0 = NPROBE + 128
        for b in range(BPC):
            _pin("act", nc.scalar.copy(
                out=csb[:, PS0 + b * 128 : PS0 + (b + 1) * 128],
                in_=pstiles[b][:]))
            lo = 0 if b == 0 else PS0 + b * 128
            hi = PS0 + (b + 1) * 128
            nc.sync.dma_start(out=out_dram[:, lo:hi], in_=csb[:, lo:hi])


_CACHE = {}


def _build():
    if "nc" in _CACHE:
        return _CACHE["nc"]
    # Bass.__init__ gpsimd-memsets four default const tensors and then
    # runs an all-engine barrier; the gpsimd engine takes ~2us just to
    # start its instruction stream, so that barrier delays the first DMA
    # kicks to ~7us.  Stub both out: the only init-const this kernel uses
    # (f32 0.0 activation bias) is remapped to a pr column in _body, and
    # cross-engine ordering is provided by Tile's semaphores.
    _orig_memset = bass.BassSharedVectorInterface.memset
    _orig_barrier = bass.Bass.all_engine_barrier
    bass.BassSharedVectorInterface.memset = lambda self, ap, v: None
    bass.Bass.all_engine_barrier = lambda self, **kw: None
    try:
        nc = bass.Bass("TRN2", debug=False)
    finally:
        bass.BassSharedVectorInterface.memset = _orig_memset
        bass.Bass.all_engine_barrier = _orig_barrier
    # Skip the kernel-tail DGE-queue drain (~3-4us); all DMAs are already
    # completion-waited by the split drains and no dynamic DMA state is
    # used.  Second-execution correctness is validated by the harness.
    type(nc.gpsimd).dma_reset = lambda self, semaphore_range=None: None
    xu_in = nc.dram_tensor("xu", [128, 2 * COLS], F16, kind="ExternalInput")
    pr_in = nc.dram_tensor("pr", [128, NPR], F32, kind="ExternalInput")
    out_d = nc.dram_tensor(
        "out", [128, NPROBE + 128 + BPC * 128], F32, kind="ExternalOutput"
    )
    with TileContext(nc) as t:
        _body(nc, t, xu_in.ap(), pr_in.ap(), out_d.ap())
    _CACHE["nc"] = nc
    return nc


def _shard_host(a):
    """[T, B, N, D] -> per-core [128, COLS] fp16, col = tc*512+b*128+n*2+d."""
    out = []
    for c in range(N_CORES):
        s = a[:, c * BPC : (c + 1) * BPC]  # [512, 4, 64, 2]
        s = s.reshape(TC, 128, BPC, N_AGENTS, D)  # (tc, p, b, n, d)
        s = np.ascontiguousarray(np.transpose(s, (1, 0, 2, 3, 4)))
        out.append(s.reshape(128, COLS).astype(np.float16))
    return out


def _probe_args():
    xp = (np.arange(128, dtype=np.float64) + 0.5) / 128.0
    ks = np.arange(NPROBE, dtype=np.float64)
    a = np.empty((128, NPR), dtype=np.float32)
    a[:, :NPROBE] = (ks[None, :] * np.pi * xp[:, None] - np.pi / 2)
    a[:, NPROBE] = -np.pi / 2
    a[:, NPROBE + 1] = -1.0
    a[:, NPROBE + 2] = 0.0
    return a


def _make_inmaps(x, u):
    xs = _shard_host(np.asarray(x, dtype=np.float32))
    us = _shard_host(np.asarray(u, dtype=np.float32))
    prb = _probe_args()
    return [
        {
            "xu": np.ascontiguousarray(np.concatenate([xs[c], us[c]], axis=1)),
            "pr": prb,
        }
        for c in range(N_CORES)
    ]


def kernel(x, u, **_):
    nc = _build()
    in_maps = _make_inmaps(x, u)
    res = bass_utils.run_bass_kernel_spmd(nc, in_maps, core_ids=list(range(N_CORES)))
    return _finish_host(res.results)


_LAST_PROBE = None


def _finish_host(outs):
    """Host reduction/unmixing in float64 -> scalar loss."""
    global _LAST_PROBE
    Cp = np.zeros((B, K_MAX, K_MAX), dtype=np.float64)
    u2 = 0.0
    PS0 = NPROBE + 128
    for c in range(N_CORES):
        o = outs[c]["out"].astype(np.float64)  # [128, 656]
        ublk = o[:, NPROBE:PS0]
        u2 += float(np.trace(ublk))
        for b in range(BPC):
            blk = o[:, PS0 + b * 128 : PS0 + (b + 1) * 128]
            acc = np.zeros((K_MAX, K_MAX))
            for nl in range(8):
                acc += blk[nl::8, nl::8]
            Cp[c * BPC + b] = acc
    o0 = outs[0]["out"].astype(np.float64)
    dev = o0[:, :NPROBE]
    tru = np.sin(_probe_args()[:, :NPROBE].astype(np.float64))
    _LAST_PROBE = np.max(np.abs(dev - tru), axis=0)

    Ct = np.einsum("ik,bkl,jl->bij", _AINV, Cp, _AINV)
    c = Ct / (_NORM[None] * (N_AGENTS * T))
    loss = np.mean((c - _COEFFS[None]) ** 2)
    loss = loss + REG * u2 / (2.0 * N_AGENTS * T * B)
    return np.array(loss, dtype=np.float32)


if __name__ == "__main__":
    rng = np.random.default_rng(0)
    x = rng.random((T, B, N_AGENTS, D), dtype=np.float32)
    u = rng.standard_normal((T, B, N_AGENTS, D)).astype(np.float32)
    print(kernel(x=x, u=u))
    print("probe err per k:", _LAST_PROBE)


# revision 31
# speedup vs baseline: 1.0524x; 1.0524x over previous
"""Ergodicity loss kernel for Trainium2 (8 NeuronCores, batch-sharded SPMD).

Math: loss = mean((c - coeffs)^2) + REG*sum(u^2)/(2*N*T*B)
      c[b,i,j] = sum_{t,n} cos(i*pi*x0)*cos(j*pi*x1) / (norm[i,j]*N*T)

Device computes, per core (4 of 32 batches; batch-sharded so no collective):
  - 16 "feature" tensors per spatial dim: fixed linear mixes of cos(k*pi*x_d)
    built from one ACT Sin (k=1), ACT Square ops (affine folded into the
    activation scale/bias; evens f2,f4,f8,f10,f12) and DVE ops (one STT for
    the clean -c3/4, plain tensor_muls for the rest).  Conditioning of the
    mixing matrix is checked row-normalized (device errors are relative).
  - Feature storage is a block layout: column = blk*256 + k*16 + nl*2 + d
    with blk = (tc*4+b)*8+oc, so matmul operands are single-free-dim
    [[2,128]] APs (the walrus verifier requires that) while feature ops see
    [[256,n],[1,16]] APs (16-elem unit-stride runs for DVE 2x_1P).
  - C'[b, k0*8+n, k1*8+n] via accumulating bf16 matmuls; off-diagonal
    n-cells are junk, dropped on the host.
  - sum(u^2) on the PE: fp16 Gram-block self-matmuls into one psum tile
    (host sums the diagonal).  Inputs arrive as fp16 (host-cast), halving
    HBM traffic; x comes as two 2 KB-per-partition-row chunks (1 KB rows
    halve effective DMA bandwidth).
  - A junk-matmul stream at kernel start keeps the PE HAM activity monitor
    busy so the PE clock is at 2.4 GHz for the u-gram / first bursts.

The Sin activation LUT is only accurate for |arg| <= ~pi/2 (measured via
the probe), hence exactly one Sin per x element and product chains for all
higher harmonics.

Host recovers the true cos-basis C by inverting the feature-mixing matrix A
(replayed symbolically in a cos-harmonic algebra), then finishes in float64.

Toolchain notes: this walrus build enforces a 1-sync-wait budget on most
instruction templates.  Structural consequences: engine-internal program
order is pinned with sync-free dep edges (Tile's scheduler otherwise
reorders and breaks the opener trick), an "opener" matmul per slab
pre-observes the ACT semaphore on the PE so real matmuls carry only the
one DVE wait their template allows, activation bias constants ride in the
"pr" input DMA (observed once by the first ACT op) instead of pre-barrier
gpsimd memsets, and the kernel-tail barrier is split into per-proc drains.
"""

import sys

sys.path.insert(0, "/opt/trn_rl_repo")

import numpy as np

import concourse.bass as bass
import concourse.mybir as mybir
from concourse import bass_utils
from concourse.tile import TileContext
from concourse.tile_rust import add_dep_helper
from concourse.vector_clock import ScopedClock, VectorClock

_orig_drain_and_barrier = TileContext._drain_and_barrier


def _split_drain_and_barrier(self, tick_clock, wait_clock):
    gc = tick_clock.global_clock
    ticks = list(gc)
    procs = [i for i, t in enumerate(ticks) if t > 0]
    for p in procs:
        vec = [0] * len(ticks)
        vec[p] = ticks[p]
        d = self.nc.sync.drain()
        wait_clock.add_sem_waits(d.ins, ScopedClock({None: VectorClock(vec)}))
    self.nc.all_engine_barrier(sem_only=True)
    popped = self.nc._tile_sem_poison_stack.pop()
    assert popped is self._sem_poison
    self.nc.clear_and_free_semaphores(list(self.sems.allocated().values()))
    self.nc.all_engine_barrier(sem_only=True)


TileContext._drain_and_barrier = _split_drain_and_barrier

# Problem constants (hardcoded per spec).
K_MAX = 16
N_AGENTS = 64
T = 512
B = 32
D = 2
REG = 1e-3
N_CORES = 8
BPC = B // N_CORES  # batches per core = 4

PI = float(np.pi)

F32 = mybir.dt.float32
F16 = mybir.dt.float16
BF16 = mybir.dt.bfloat16

# Per-core geometry: x shard [T=512, BPC=4, N=64, D=2] is host-permuted to
# [128, 2048] fp16 with partition p = t % 128 and column
# tc*512 + b*128 + n*2 + d  (tc = t // 128).
TC = 4
COLS = TC * BPC * N_AGENTS * D  # 2048
NBLK = 128  # feature blocks (tc, b, oc)
NPROBE = 16
NPR = NPROBE + 3  # probe cols + {-pi/2, -1.0, 0.0} activation-bias consts




# ---------------------------------------------------------------------------
# Symbolic harmonic algebra.
# ---------------------------------------------------------------------------
class Harm:
    __slots__ = ("c",)

    def __init__(self, c):
        self.c = np.asarray(c, dtype=np.float64)

    @staticmethod
    def const(v):
        c = np.zeros(K_MAX)
        c[0] = v
        return Harm(c)

    @staticmethod
    def basis(k, v=1.0):
        c = np.zeros(K_MAX)
        c[k] = v
        return Harm(c)

    def affine(self, scale, bias):
        c = self.c * scale
        c[0] += bias
        return Harm(c)

    def mul(self, other):
        out = np.zeros(K_MAX)
        for a in range(K_MAX):
            if self.c[a] == 0.0:
                continue
            for b in range(K_MAX):
                if other.c[b] == 0.0:
                    continue
                v = self.c[a] * other.c[b]
                s, d = a + b, abs(a - b)
                assert s < K_MAX or v == 0.0, f"harmonic overflow {a}+{b}"
                out[s] += 0.5 * v
                out[d] += 0.5 * v
        return Harm(out)

    def square(self, scale=1.0, bias=0.0):
        z = self.affine(scale, bias)
        return z.mul(z)

    def stt(self, s, other):  # (self - s) * other
        return self.affine(1.0, -s).mul(other)


def _feature_mixing_matrix():
    """Replay the device feature pipeline symbolically -> A[16,16].
    Must mirror the ops in _body exactly."""
    f = [None] * K_MAX
    f[0] = Harm.const(1.0)
    f[1] = Harm.basis(1, -1.0)      # ACT Sin(pi*x - pi/2) = -cos(pi*x)
    f[2] = f[1].square()             # ACT
    f[4] = f[2].square(2.0, -1.0)    # ACT
    f[8] = f[4].square(2.0, -1.0)    # ACT
    f[3] = f[2].stt(0.75, f[1])      # DVE STT -> -c3/4
    f[6] = f[3].mul(f[3])            # DVE TT -> (c6+1)/32
    f[5] = f[4].mul(f[1])            # DVE TT
    f[7] = f[6].mul(f[1])            # DVE TT
    f[10] = f[5].square(4.0, 0.0)    # ACT
    f[12] = f[6].square(32.0, -1.0)  # ACT -> (c12+1)/2
    f[9] = f[8].mul(f[1])            # DVE TT
    f[14] = f[7].mul(f[7])           # DVE TT
    f[11] = f[10].mul(f[1])          # DVE leaf
    f[13] = f[12].mul(f[1])          # DVE leaf
    f[15] = f[14].mul(f[1])          # DVE leaf
    return np.stack([x.c for x in f])


_A = _feature_mixing_matrix()
_AINV = np.linalg.inv(_A)
_ROWCOND = np.linalg.cond(_A / np.linalg.norm(_A, axis=1, keepdims=True))
assert _ROWCOND < 1e3, _ROWCOND


def _np_constants():
    ks = np.arange(K_MAX, dtype=np.float64)
    vs = []
    for _ in range(D):
        with np.errstate(divide="ignore", invalid="ignore"):
            ki = ks * np.pi
            nz = (np.exp(1j * ki) - 1.0) / (1j * ki)
        integral = np.where(ks == 0, 1.0 + 0j, nz)
        vs.append(integral)
    cd = np.real(vs[0][:, None] * vs[1][None, :]).astype(np.float64)
    norm_last = np.where(ks == 0, 1.0, np.sqrt(0.5))
    norm = np.broadcast_to(norm_last[None, :], (K_MAX, K_MAX)).copy()
    return cd / norm, norm


_COEFFS, _NORM = _np_constants()


# ---------------------------------------------------------------------------
# Device program
# ---------------------------------------------------------------------------
def _body(nc, tc, xu_in, pr_in, out_dram):
    Sq = mybir.ActivationFunctionType.Square
    Sin = mybir.ActivationFunctionType.Sin
    sub = mybir.AluOpType.subtract
    mult = mybir.AluOpType.mult

    with (
        tc.tile_pool(name="io", bufs=1) as io_pool,
        tc.tile_pool(name="feat", bufs=1) as feat_pool,
        tc.tile_pool(name="work", bufs=1) as work_pool,
        tc.tile_pool(name="psum", bufs=1, space="PSUM") as psum_pool,
    ):
        xu = io_pool.tile([128, 2 * COLS], F16, tag="xu")
        pr = io_pool.tile([128, NPR], F32, tag="pr")
        # Activation bias consts ride in the pr DMA; the probe Sin (first
        # ACT op) observes that queue once for the whole ACT stream.
        nc.const_aps.aps[(F32, -PI / 2)] = pr[:, NPROBE : NPROBE + 1]
        nc.const_aps.aps[(F32, -1.0)] = pr[:, NPROBE + 1 : NPROBE + 2]
        nc.const_aps.aps[(F32, 0.0)] = pr[:, NPROBE + 2 : NPROBE + 3]

        # DMA kicks (each dma_start = one HW queue, kick instructions
        # serialize ~0.65us each on SP): pr FIRST — its tiny-row transfer
        # finishes in ~1.4us when kicked before the 1 MB x/u flood, but
        # gets starved to ~14us behind it (and the probe gates every
        # bias-const-reading ACT op).
        nc.sync.dma_start(out=pr[:], in_=pr_in[:])
        HC = COLS // 2
        nc.sync.dma_start(out=xu[:, 0:HC], in_=xu_in[:, 0:HC])
        nc.sync.dma_start(out=xu[:, HC:COLS], in_=xu_in[:, HC:COLS])
        nc.sync.dma_start(out=xu[:, COLS:], in_=xu_in[:, COLS:])
        uraw = xu[:, COLS : 2 * COLS]

        # Feature storage block layout (see module docstring).
        FA = feat_pool.tile([128, K_MAX * COLS], BF16, tag="FA")
        FAke = FA[:].rearrange(
            "p (blk k e) -> p k blk e", blk=NBLK, k=K_MAX, e=16
        )
        FAm = FA[:].rearrange(
            "p (blk kn d) -> p blk d kn", blk=NBLK, kn=128, d=D
        )

        def F(k, b0=0, b1=NBLK):
            return FAke[:, k, b0:b1]

        warm = work_pool.tile([128, 128], BF16, tag="warm")
        # csb layout: [probe 16][psu 128][ps0..ps3 512] so each output DMA
        # covers one contiguous span and the b3 tail DMA is small.
        csb = work_pool.tile([128, NPROBE + 128 + BPC * 128], F32, tag="csb")

        pstiles = [
            psum_pool.tile([128, 128], F32, tag=f"ps{b}", name=f"ps{b}")
            for b in range(BPC)
        ]
        psu = psum_pool.tile([128, 128], F32, tag="psu")
        pjunk = psum_pool.tile([128, 128], F32, tag="pjunk")

        # Pin engine-internal program order (Tile reorders otherwise).
        _last = {}

        def _pin(key, bi):
            if key in _last:
                add_dep_helper(bi.ins, _last[key].ins, sync=False,
                               reason=f"{key} order pin")
            _last[key] = bi
            return bi

        # GPSIMD: f0 = ones first, warm second; the first junk matmul's
        # single gpsimd wait covers both.
        _pin("gp", nc.gpsimd.memset(F(0), 1.0))
        _pin("gp", nc.gpsimd.memset(warm[:], 1.0))

        HB = NBLK // 2
        SL = {0: (0, HB), 1: (HB, NBLK)}

        def act(out, in_, func, **kw):
            return _pin("act", nc.scalar.activation(out, in_, func, **kw))

        def stt(k_out, k_in, s, k_mul, sl):
            b0, b1 = sl
            return _pin("dve", nc.vector.scalar_tensor_tensor(
                F(k_out, b0, b1), F(k_in, b0, b1), s, F(k_mul, b0, b1),
                sub, mult,
            ))

        def tt(k_out, k_a, k_b, sl):
            b0, b1 = sl
            return _pin("dve", nc.vector.tensor_mul(
                out=F(k_out, b0, b1), in0=F(k_a, b0, b1), in1=F(k_b, b0, b1)
            ))

        def sq(k_out, k_in, sl, scale=1.0, bias=0.0):
            b0, b1 = sl
            return act(F(k_out, b0, b1), F(k_in, b0, b1), Sq,
                       scale=scale, bias=bias)

        # --- ACT + DVE streams ---
        # Issue order MUST be topological (producer before consumer): Tile
        # tracks dataflow by issue order; a consumer issued before its
        # producer reads "uninitialized" bytes and the real write becomes a
        # reversed WAR anti-dep.  Per-engine execution order = issue order
        # (pinned above).  ACT order is by downstream urgency: the leaf
        # muls (f11, f13) that gate the matmul bursts need f10/f12 as early
        # as their DVE inputs allow.
        act(csb[:, :NPROBE], pr[:, :NPROBE], Sin, scale=1.0)         # probe
        act(F(1, 0, HB), xu[:, 0:HC], Sin, scale=PI, bias=-PI / 2)   # sin s0
        act(F(1, HB, NBLK), xu[:, HC:COLS], Sin, scale=PI, bias=-PI / 2)
        sq(2, 1, SL[0])
        sq(2, 1, SL[1])
        stt(3, 2, 0.75, 1, SL[0])
        stt(3, 2, 0.75, 1, SL[1])
        sq(4, 2, SL[0], 2.0, -1.0)
        sq(4, 2, SL[1], 2.0, -1.0)
        tt(6, 3, 3, SL[0])
        tt(6, 3, 3, SL[1])
        sq(8, 4, SL[0], 2.0, -1.0)
        tt(5, 4, 1, SL[0])
        tt(7, 6, 1, SL[0])
        tt(14, 7, 7, SL[0])
        sq(12, 6, SL[0], 32.0, -1.0)
        sq(10, 5, SL[0], 4.0, 0.0)
        sq(8, 4, SL[1], 2.0, -1.0)
        tt(5, 4, 1, SL[1])
        tt(7, 6, 1, SL[1])
        tt(14, 7, 7, SL[1])
        sq(12, 6, SL[1], 32.0, -1.0)
        sq(10, 5, SL[1], 4.0, 0.0)
        tt(9, 8, 1, SL[0])
        # slab-0 leaves, per tc so the tc0 burst starts early
        for tcc in (0, 1):
            hs = (tcc * 32, (tcc + 1) * 32)
            tt(15, 14, 1, hs)
            tt(11, 10, 1, hs)
            tt(13, 12, 1, hs)
        tt(9, 8, 1, SL[1])
        for tcc in (2, 3):
            hs = (tcc * 32, (tcc + 1) * 32)
            tt(15, 14, 1, hs)
            tt(11, 10, 1, hs)
            tt(13, 12, 1, hs)

        # --- PE stream ---
        last_mm = None

        def junk(n):
            # 1-column matmuls: keep the HAM activity monitor busy with
            # ~no SBUF traffic (full-width junk matmuls stream 64 KB each
            # and visibly steal SBUF bandwidth from ACT/DVE).
            nonlocal last_mm
            for _ in range(n):
                mm = nc.tensor.matmul(
                    pjunk[0:1, 100:101], warm[:, 0:1], warm[:, 0:1],
                    start=True, stop=True, skip_group_check=True,
                )
                if last_mm is not None:
                    add_dep_helper(mm.ins, last_mm.ins, sync=False,
                                   reason="junk chain")
                last_mm = mm

        for c in range(16):
            ub = uraw[:, c * 128 : (c + 1) * 128]
            mm = nc.tensor.matmul(
                psu[:], ub, ub, start=(c == 0), stop=(c == 15),
                skip_group_check=True,
            )
            if last_mm is not None:
                add_dep_helper(mm.ins, last_mm.ins, sync=False, reason="ugram")
            last_mm = mm
        # Warm the PE clock for the real bursts: the HAM un-throttles only
        # after ~3.4us of SUSTAINED busy, so tiny keep-alive matmuls are
        # not enough.  Run a ~3.5us stream of 32-col junk matmuls gated on
        # f8 of slab 1 (~3.5us before the first burst); 32-col operands
        # keep the SBUF read traffic low enough not to rob ACT/DVE
        # (full-width junk measurably slowed every ACT op by ~20%).
        stub = FAke[:, 8, HB - 1 : HB, 15:16]  # f8 slab 0: ~3.5us pre-burst
        gate = nc.tensor.matmul(
            pjunk[0:1, 110:111], stub, stub,
            start=True, stop=True, skip_group_check=True,
        )
        add_dep_helper(gate.ins, last_mm.ins, sync=False, reason="warm gate")
        last_mm = gate
        for _ in range(60):
            mm = nc.tensor.matmul(
                pjunk[0:32, 64:96], warm[:, 0:32], warm[:, 0:32],
                start=True, stop=True, skip_group_check=True,
            )
            add_dep_helper(mm.ins, last_mm.ins, sync=False, reason="warm32")
            last_mm = mm

        # Real bursts: per slab an opener matmul reads the slab's LAST
        # ACT-written feature (f10, per the pinned ACT order) so real
        # Ldweights carry only their single DVE wait.
        for si in (0, 1):
            b0, b1 = SL[si]
            fstub = FAke[:, 10, b1 - 1 : b1, 15:16]
            op = nc.tensor.matmul(
                pjunk[0:1, 120 + si : 121 + si], fstub, fstub,
                start=True, stop=True, skip_group_check=True,
            )
            add_dep_helper(op.ins, last_mm.ins, sync=False,
                           reason="opener after prev MMs")
            opener = op
            last_mm = op
            for tcc in (2 * si, 2 * si + 1):
                for b in range(BPC):
                    ps = pstiles[b]
                    for oc in range(8):
                        blk = (tcc * BPC + b) * 8 + oc
                        mm = nc.tensor.matmul(
                            ps[:], FAm[:, blk, 0], FAm[:, blk, 1],
                            start=(tcc == 0 and oc == 0),
                            stop=(tcc == TC - 1 and oc == 7),
                            skip_group_check=True,
                        )
                        add_dep_helper(mm.ins, opener.ins, sync=False,
                                       reason="PE wait-slot opener")
                        last_mm = mm

        # PSUM -> SBUF on ACT; one output DMA per contiguous span so the
        # output transfer overlaps the burst tail instead of serializing
        # 328 KB after the last copy.  probe+psu ride with b0's DMA to stay
        # within the 8 HW queues (4 in + 4 out).
        _pin("act", nc.scalar.copy(
            out=csb[:, NPROBE : NPROBE + 128], in_=psu[:]))
        PS0 = NPROBE + 128
        for b in range(BPC):
            _pin("act", nc.scalar.copy(
                out=csb[:, PS0 + b * 128 : PS0 + (b + 1) * 128],
                in_=pstiles[b][:]))
            lo = 0 if b == 0 else PS0 + b * 128
            hi = PS0 + (b + 1) * 128
            nc.sync.dma_start(out=out_dram[:, lo:hi], in_=csb[:, lo:hi])


_CACHE = {}


def _build():
    if "nc" in _CACHE:
        return _CACHE["nc"]
    # Bass.__init__ gpsimd-memsets four default const tensors and then
    # runs an all-engine barrier; the gpsimd engine takes ~2us just to
    # start its instruction stream, so that barrier delays the first DMA
    # kicks to ~7us.  Stub both out: the only init-const this kernel uses
    # (f32 0.0 activation bias) is remapped to a pr column in _body, and
    # cross-engine ordering is provided by Tile's semaphores.
    _orig_memset = bass.BassSharedVectorInterface.memset
    _orig_barrier = bass.Bass.all_engine_barrier
    bass.BassSharedVectorInterface.memset = lambda self, ap, v: None
    bass.Bass.all_engine_barrier = lambda self, **kw: None
    try:
        nc = bass.Bass("TRN2", debug=False)
    finally:
        bass.BassSharedVectorInterface.memset = _orig_memset
        bass.Bass.all_engine_barrier = _orig_barrier
    # Skip the kernel-tail DGE-queue drain (~3-4us); all DMAs are already
    # completion-waited by the split drains and no dynamic DMA state is
    # used.  Second-execution correctness is validated by the harness.
    type(nc.gpsimd).dma_reset = lambda self, semaphore_range=None: None
    xu_in = nc.dram_tensor("xu", [128, 2 * COLS], F16, kind="ExternalInput")
    pr_in = nc.dram_tensor("pr", [128, NPR], F32, kind="ExternalInput")
    out_d = nc.dram_tensor(
        "out", [128, NPROBE + 128 + BPC * 128], F32, kind="ExternalOutput"
    )
    with TileContext(nc) as t:
        _body(nc, t, xu_in.ap(), pr_in.ap(), out_d.ap())
    _CACHE["nc"] = nc
    return nc


def _shard_host(a):
    """[T, B, N, D] -> per-core [128, COLS] fp16, col = tc*512+b*128+n*2+d."""
    out = []
    for c in range(N_CORES):
        s = a[:, c * BPC : (c + 1) * BPC]  # [512, 4, 64, 2]
        s = s.reshape(TC, 128, BPC, N_AGENTS, D)  # (tc, p, b, n, d)
        s = np.ascontiguousarray(np.transpose(s, (1, 0, 2, 3, 4)))
        out.append(s.reshape(128, COLS).astype(np.float16))
    return out


def _probe_args():
    xp = (np.arange(128, dtype=np.float64) + 0.5) / 128.0
    ks = np.arange(NPROBE, dtype=np.float64)
    a = np.empty((128, NPR), dtype=np.float32)
    a[:, :NPROBE] = (ks[None, :] * np.pi * xp[:, None] - np.pi / 2)
    a[:, NPROBE] = -np.pi / 2
    a[:, NPROBE + 1] = -1.0
    a[:, NPROBE + 2] = 0.0
    return a


def _make_inmaps(x, u):
    xs = _shard_host(np.asarray(x, dtype=np.float32))
    us = _shard_host(np.asarray(u, dtype=np.float32))
    prb = _probe_args()
    return [
        {
            "xu": np.ascontiguousarray(np.concatenate([xs[c], us[c]], axis=1)),
            "pr": prb,
        }
        for c in range(N_CORES)
    ]


def kernel(x, u, **_):
    nc = _build()
    in_maps = _make_inmaps(x, u)
    res = bass_utils.run_bass_kernel_spmd(nc, in_maps, core_ids=list(range(N_CORES)))
    return _finish_host(res.results)


_LAST_PROBE = None


def _finish_host(outs):
    """Host reduction/unmixing in float64 -> scalar loss."""
    global _LAST_PROBE
    Cp = np.zeros((B, K_MAX, K_MAX), dtype=np.float64)
    u2 = 0.0
    PS0 = NPROBE + 128
    for c in range(N_CORES):
        o = outs[c]["out"].astype(np.float64)  # [128, 656]
        ublk = o[:, NPROBE:PS0]
        u2 += float(np.trace(ublk))
        for b in range(BPC):
            blk = o[:, PS0 + b * 128 : PS0 + (b + 1) * 128]
            acc = np.zeros((K_MAX, K_MAX))
            for nl in range(8):
                acc += blk[nl::8, nl::8]
            Cp[c * BPC + b] = acc
    o0 = outs[0]["out"].astype(np.float64)
    dev = o0[:, :NPROBE]
    tru = np.sin(_probe_args()[:, :NPROBE].astype(np.float64))
    _LAST_PROBE = np.max(np.abs(dev - tru), axis=0)

    Ct = np.einsum("ik,bkl,jl->bij", _AINV, Cp, _AINV)
    c = Ct / (_NORM[None] * (N_AGENTS * T))
    loss = np.mean((c - _COEFFS[None]) ** 2)
    loss = loss + REG * u2 / (2.0 * N_AGENTS * T * B)
    return np.array(loss, dtype=np.float32)


if __name__ == "__main__":
    rng = np.random.default_rng(0)
    x = rng.random((T, B, N_AGENTS, D), dtype=np.float32)
    u = rng.standard_normal((T, B, N_AGENTS, D)).astype(np.float32)
    print(kernel(x=x, u=u))
    print("probe err per k:", _LAST_PROBE)


# revision 33
# speedup vs baseline: 1.1239x; 1.0680x over previous
"""Ergodicity loss kernel for Trainium2 (8 NeuronCores, batch-sharded SPMD).

Math: loss = mean((c - coeffs)^2) + REG*sum(u^2)/(2*N*T*B)
      c[b,i,j] = sum_{t,n} cos(i*pi*x0)*cos(j*pi*x1) / (norm[i,j]*N*T)

Device computes, per core (4 of 32 batches; batch-sharded so no collective):
  - 16 "feature" tensors per spatial dim: fixed linear mixes of cos(k*pi*x_d)
    built from one ACT Sin (k=1), ACT Square ops (affine folded into the
    activation scale/bias; evens f2,f4,f8,f10,f12) and DVE ops (one STT for
    the clean -c3/4, plain tensor_muls for the rest).  Conditioning of the
    mixing matrix is checked row-normalized (device errors are relative).
  - Feature storage is a block layout: column = blk*256 + k*16 + nl*2 + d
    with blk = (tc*4+b)*8+oc, so matmul operands are single-free-dim
    [[2,128]] APs (the walrus verifier requires that) while feature ops see
    [[256,n],[1,16]] APs (16-elem unit-stride runs for DVE 2x_1P).
  - C'[b, k0*8+n, k1*8+n] via accumulating bf16 matmuls; off-diagonal
    n-cells are junk, dropped on the host.
  - sum(u^2) on the PE: fp16 Gram-block self-matmuls into one psum tile
    (host sums the diagonal).  Inputs arrive as fp16 (host-cast), halving
    HBM traffic; x comes as two 2 KB-per-partition-row chunks (1 KB rows
    halve effective DMA bandwidth).
  - A junk-matmul stream at kernel start keeps the PE HAM activity monitor
    busy so the PE clock is at 2.4 GHz for the u-gram / first bursts.

The Sin activation LUT is only accurate for |arg| <= ~pi/2 (measured via
the probe), hence exactly one Sin per x element and product chains for all
higher harmonics.

Host recovers the true cos-basis C by inverting the feature-mixing matrix A
(replayed symbolically in a cos-harmonic algebra), then finishes in float64.

Toolchain notes: this walrus build enforces a 1-sync-wait budget on most
instruction templates.  Structural consequences: engine-internal program
order is pinned with sync-free dep edges (Tile's scheduler otherwise
reorders and breaks the opener trick), an "opener" matmul per slab
pre-observes the ACT semaphore on the PE so real matmuls carry only the
one DVE wait their template allows, activation bias constants ride in the
"pr" input DMA (observed once by the first ACT op) instead of pre-barrier
gpsimd memsets, and the kernel-tail barrier is split into per-proc drains.
"""

import sys

sys.path.insert(0, "/opt/trn_rl_repo")

import numpy as np

import concourse.bass as bass
import concourse.mybir as mybir
from concourse import bass_utils
from concourse.tile import TileContext
from concourse.tile_rust import add_dep_helper
from concourse.vector_clock import ScopedClock, VectorClock

_orig_drain_and_barrier = TileContext._drain_and_barrier


def _split_drain_and_barrier(self, tick_clock, wait_clock):
    gc = tick_clock.global_clock
    ticks = list(gc)
    procs = [i for i, t in enumerate(ticks) if t > 0]
    for p in procs:
        vec = [0] * len(ticks)
        vec[p] = ticks[p]
        d = self.nc.sync.drain()
        wait_clock.add_sem_waits(d.ins, ScopedClock({None: VectorClock(vec)}))
    self.nc.all_engine_barrier(sem_only=True)
    popped = self.nc._tile_sem_poison_stack.pop()
    assert popped is self._sem_poison
    self.nc.clear_and_free_semaphores(list(self.sems.allocated().values()))
    self.nc.all_engine_barrier(sem_only=True)


TileContext._drain_and_barrier = _split_drain_and_barrier

# Problem constants (hardcoded per spec).
K_MAX = 16
N_AGENTS = 64
T = 512
B = 32
D = 2
REG = 1e-3
N_CORES = 8
BPC = B // N_CORES  # batches per core = 4

PI = float(np.pi)

F32 = mybir.dt.float32
F16 = mybir.dt.float16
BF16 = mybir.dt.bfloat16

# Per-core geometry: x shard [T=512, BPC=4, N=64, D=2] is host-permuted to
# [128, 2048] fp16 with partition p = t % 128 and column
# tc*512 + b*128 + n*2 + d  (tc = t // 128).
TC = 4
COLS = TC * BPC * N_AGENTS * D  # 2048
NBLK = 128  # feature blocks (tc, b, oc)
NPROBE = 16
NPR = NPROBE + 3  # probe cols + {-pi/2, -1.0, 0.0} activation-bias consts




# ---------------------------------------------------------------------------
# Symbolic harmonic algebra.
# ---------------------------------------------------------------------------
class Harm:
    __slots__ = ("c",)

    def __init__(self, c):
        self.c = np.asarray(c, dtype=np.float64)

    @staticmethod
    def const(v):
        c = np.zeros(K_MAX)
        c[0] = v
        return Harm(c)

    @staticmethod
    def basis(k, v=1.0):
        c = np.zeros(K_MAX)
        c[k] = v
        return Harm(c)

    def affine(self, scale, bias):
        c = self.c * scale
        c[0] += bias
        return Harm(c)

    def mul(self, other):
        out = np.zeros(K_MAX)
        for a in range(K_MAX):
            if self.c[a] == 0.0:
                continue
            for b in range(K_MAX):
                if other.c[b] == 0.0:
                    continue
                v = self.c[a] * other.c[b]
                s, d = a + b, abs(a - b)
                assert s < K_MAX or v == 0.0, f"harmonic overflow {a}+{b}"
                out[s] += 0.5 * v
                out[d] += 0.5 * v
        return Harm(out)

    def square(self, scale=1.0, bias=0.0):
        z = self.affine(scale, bias)
        return z.mul(z)

    def stt(self, s, other):  # (self - s) * other
        return self.affine(1.0, -s).mul(other)


def _feature_mixing_matrix():
    """Replay the device feature pipeline symbolically -> A[16,16].
    Must mirror the ops in _body exactly."""
    f = [None] * K_MAX
    f[0] = Harm.const(1.0)
    f[1] = Harm.basis(1, -1.0)      # ACT Sin(pi*x - pi/2) = -cos(pi*x)
    f[2] = f[1].square()             # ACT
    f[4] = f[2].square(2.0, -1.0)    # ACT
    f[8] = f[4].square(2.0, -1.0)    # ACT
    f[3] = f[2].stt(0.75, f[1])      # DVE STT -> -c3/4
    f[6] = f[3].mul(f[3])            # DVE TT -> (c6+1)/32
    f[5] = f[4].mul(f[1])            # DVE TT
    f[7] = f[6].mul(f[1])            # DVE TT
    f[10] = f[5].square(4.0, 0.0)    # ACT
    f[12] = f[6].square(32.0, -1.0)  # ACT -> (c12+1)/2
    f[9] = f[8].mul(f[1])            # DVE TT
    f[14] = f[7].mul(f[7])           # DVE TT
    f[11] = f[10].mul(f[1])          # DVE leaf
    f[13] = f[12].mul(f[1])          # DVE leaf
    f[15] = f[14].mul(f[1])          # DVE leaf
    return np.stack([x.c for x in f])


_A = _feature_mixing_matrix()
_AINV = np.linalg.inv(_A)
_ROWCOND = np.linalg.cond(_A / np.linalg.norm(_A, axis=1, keepdims=True))
assert _ROWCOND < 1e3, _ROWCOND


def _np_constants():
    ks = np.arange(K_MAX, dtype=np.float64)
    vs = []
    for _ in range(D):
        with np.errstate(divide="ignore", invalid="ignore"):
            ki = ks * np.pi
            nz = (np.exp(1j * ki) - 1.0) / (1j * ki)
        integral = np.where(ks == 0, 1.0 + 0j, nz)
        vs.append(integral)
    cd = np.real(vs[0][:, None] * vs[1][None, :]).astype(np.float64)
    norm_last = np.where(ks == 0, 1.0, np.sqrt(0.5))
    norm = np.broadcast_to(norm_last[None, :], (K_MAX, K_MAX)).copy()
    return cd / norm, norm


_COEFFS, _NORM = _np_constants()


# ---------------------------------------------------------------------------
# Device program
# ---------------------------------------------------------------------------
def _body(nc, tc, xu_in, pr_in, out_dram):
    Sq = mybir.ActivationFunctionType.Square
    Sin = mybir.ActivationFunctionType.Sin
    sub = mybir.AluOpType.subtract
    mult = mybir.AluOpType.mult

    with (
        tc.tile_pool(name="io", bufs=1) as io_pool,
        tc.tile_pool(name="feat", bufs=1) as feat_pool,
        tc.tile_pool(name="work", bufs=1) as work_pool,
        tc.tile_pool(name="psum", bufs=1, space="PSUM") as psum_pool,
    ):
        xu = io_pool.tile([128, 2 * COLS], F16, tag="xu")
        pr = io_pool.tile([128, NPR], F32, tag="pr")
        # Activation bias consts ride in the pr DMA; the probe Sin (first
        # ACT op) observes that queue once for the whole ACT stream.
        nc.const_aps.aps[(F32, -PI / 2)] = pr[:, NPROBE : NPROBE + 1]
        nc.const_aps.aps[(F32, -1.0)] = pr[:, NPROBE + 1 : NPROBE + 2]
        nc.const_aps.aps[(F32, 0.0)] = pr[:, NPROBE + 2 : NPROBE + 3]

        # DMA kicks (each dma_start = one HW queue, kick instructions
        # serialize ~0.65us each on SP): pr FIRST — its tiny-row transfer
        # finishes in ~1.4us when kicked before the 1 MB x/u flood, but
        # gets starved to ~14us behind it (and the probe gates every
        # bias-const-reading ACT op).
        nc.sync.dma_start(out=pr[:], in_=pr_in[:])
        HC = COLS // 2
        nc.sync.dma_start(out=xu[:, 0:HC], in_=xu_in[:, 0:HC])
        nc.sync.dma_start(out=xu[:, HC:COLS], in_=xu_in[:, HC:COLS])
        nc.sync.dma_start(out=xu[:, COLS:], in_=xu_in[:, COLS:])
        uraw = xu[:, COLS : 2 * COLS]

        # Feature storage block layout (see module docstring).
        FA = feat_pool.tile([128, K_MAX * COLS], BF16, tag="FA")
        FAke = FA[:].rearrange(
            "p (blk k e) -> p k blk e", blk=NBLK, k=K_MAX, e=16
        )
        FAm = FA[:].rearrange(
            "p (blk kn d) -> p blk d kn", blk=NBLK, kn=128, d=D
        )

        def F(k, b0=0, b1=NBLK):
            return FAke[:, k, b0:b1]

        warm = work_pool.tile([128, 128], BF16, tag="warm")
        # csb layout: [probe 16][psu 128][ps0..ps3 512] so each output DMA
        # covers one contiguous span and the b3 tail DMA is small.
        csb = work_pool.tile([128, NPROBE + 128 + BPC * 128], F32, tag="csb")

        pstiles = [
            psum_pool.tile([128, 128], F32, tag=f"ps{b}", name=f"ps{b}")
            for b in range(BPC)
        ]
        psu = psum_pool.tile([128, 128], F32, tag="psu")
        pjunk = psum_pool.tile([128, 128], F32, tag="pjunk")

        # Pin engine-internal program order (Tile reorders otherwise).
        _last = {}

        def _pin(key, bi):
            if key in _last:
                add_dep_helper(bi.ins, _last[key].ins, sync=False,
                               reason=f"{key} order pin")
            _last[key] = bi
            return bi

        # GPSIMD: f0 = ones first, warm second; the first junk matmul's
        # single gpsimd wait covers both.
        _pin("gp", nc.gpsimd.memset(F(0), 1.0))
        _pin("gp", nc.gpsimd.memset(warm[:], 1.0))

        HB = NBLK // 2
        SL = {0: (0, HB), 1: (HB, NBLK)}

        def act(out, in_, func, **kw):
            return _pin("act", nc.scalar.activation(out, in_, func, **kw))

        def stt(k_out, k_in, s, k_mul, sl):
            b0, b1 = sl
            return _pin("dve", nc.vector.scalar_tensor_tensor(
                F(k_out, b0, b1), F(k_in, b0, b1), s, F(k_mul, b0, b1),
                sub, mult,
            ))

        def tt(k_out, k_a, k_b, sl):
            b0, b1 = sl
            return _pin("dve", nc.vector.tensor_mul(
                out=F(k_out, b0, b1), in0=F(k_a, b0, b1), in1=F(k_b, b0, b1)
            ))

        def sq(k_out, k_in, sl, scale=1.0, bias=0.0):
            b0, b1 = sl
            return act(F(k_out, b0, b1), F(k_in, b0, b1), Sq,
                       scale=scale, bias=bias)

        # --- ACT + DVE streams ---
        # Issue order MUST be topological (producer before consumer): Tile
        # tracks dataflow by issue order; a consumer issued before its
        # producer reads "uninitialized" bytes and the real write becomes a
        # reversed WAR anti-dep.  Per-engine execution order = issue order
        # (pinned above).  ACT order is by downstream urgency: the leaf
        # muls (f11, f13) that gate the matmul bursts need f10/f12 as early
        # as their DVE inputs allow.
        # Slab-0-FIRST schedule: the matmul bursts are a pure PE tail, so
        # completing slab 0's whole feature set as early as possible (all
        # slab-1 work deferred) lets the first 64-matmul burst overlap the
        # slab-1 feature phase entirely.
        act(csb[:, :NPROBE], pr[:, :NPROBE], Sin, scale=1.0)         # probe
        act(F(1, 0, HB), xu[:, 0:HC], Sin, scale=PI, bias=-PI / 2)   # sin s0
        sq(2, 1, SL[0])
        stt(3, 2, 0.75, 1, SL[0])
        sq(4, 2, SL[0], 2.0, -1.0)
        tt(6, 3, 3, SL[0])
        tt(5, 4, 1, SL[0])
        sq(8, 4, SL[0], 2.0, -1.0)
        tt(7, 6, 1, SL[0])
        tt(14, 7, 7, SL[0])
        tt(9, 8, 1, SL[0])
        sq(12, 6, SL[0], 32.0, -1.0)
        sq(10, 5, SL[0], 4.0, 0.0)
        for tcc in (0, 1):
            hs = (tcc * 32, (tcc + 1) * 32)
            tt(15, 14, 1, hs)
            tt(11, 10, 1, hs)
            tt(13, 12, 1, hs)
        # slab 1
        act(F(1, HB, NBLK), xu[:, HC:COLS], Sin, scale=PI, bias=-PI / 2)
        sq(2, 1, SL[1])
        stt(3, 2, 0.75, 1, SL[1])
        sq(4, 2, SL[1], 2.0, -1.0)
        tt(6, 3, 3, SL[1])
        tt(5, 4, 1, SL[1])
        sq(8, 4, SL[1], 2.0, -1.0)
        tt(7, 6, 1, SL[1])
        tt(14, 7, 7, SL[1])
        tt(9, 8, 1, SL[1])
        sq(12, 6, SL[1], 32.0, -1.0)
        sq(10, 5, SL[1], 4.0, 0.0)
        for tcc in (2, 3):
            hs = (tcc * 32, (tcc + 1) * 32)
            tt(15, 14, 1, hs)
            tt(11, 10, 1, hs)
            tt(13, 12, 1, hs)

        # --- PE stream ---
        last_mm = None

        def junk(n):
            # 1-column matmuls: keep the HAM activity monitor busy with
            # ~no SBUF traffic (full-width junk matmuls stream 64 KB each
            # and visibly steal SBUF bandwidth from ACT/DVE).
            nonlocal last_mm
            for _ in range(n):
                mm = nc.tensor.matmul(
                    pjunk[0:1, 100:101], warm[:, 0:1], warm[:, 0:1],
                    start=True, stop=True, skip_group_check=True,
                )
                if last_mm is not None:
                    add_dep_helper(mm.ins, last_mm.ins, sync=False,
                                   reason="junk chain")
                last_mm = mm

        for c in range(16):
            ub = uraw[:, c * 128 : (c + 1) * 128]
            mm = nc.tensor.matmul(
                psu[:], ub, ub, start=(c == 0), stop=(c == 15),
                skip_group_check=True,
            )
            if last_mm is not None:
                add_dep_helper(mm.ins, last_mm.ins, sync=False, reason="ugram")
            last_mm = mm
        # Warm the PE clock for the real bursts: the HAM un-throttles only
        # after ~3.4us of SUSTAINED busy, so tiny keep-alive matmuls are
        # not enough.  Run a ~3.5us stream of 32-col junk matmuls gated on
        # f8 of slab 1 (~3.5us before the first burst); 32-col operands
        # keep the SBUF read traffic low enough not to rob ACT/DVE
        # (full-width junk measurably slowed every ACT op by ~20%).
        stub = FAke[:, 8, HB - 1 : HB, 15:16]  # f8 slab 0: ~3.5us pre-burst
        gate = nc.tensor.matmul(
            pjunk[0:1, 110:111], stub, stub,
            start=True, stop=True, skip_group_check=True,
        )
        add_dep_helper(gate.ins, last_mm.ins, sync=False, reason="warm gate")
        last_mm = gate
        for _ in range(60):
            mm = nc.tensor.matmul(
                pjunk[0:32, 64:96], warm[:, 0:32], warm[:, 0:32],
                start=True, stop=True, skip_group_check=True,
            )
            add_dep_helper(mm.ins, last_mm.ins, sync=False, reason="warm32")
            last_mm = mm

        # Real bursts: per slab an opener matmul reads the slab's LAST
        # ACT-written feature (f10, per the pinned ACT order) so real
        # Ldweights carry only their single DVE wait.  Between the slab-0
        # and slab-1 bursts the PE idles ~3us waiting on slab-1 leaves; a
        # second small gated warm stream bridges that window.
        for si in (0, 1):
            b0, b1 = SL[si]
            if si == 1:
                stub2 = FAke[:, 12, NBLK - 1 : NBLK, 15:16]
                g2 = nc.tensor.matmul(
                    pjunk[0:1, 112:113], stub2, stub2,
                    start=True, stop=True, skip_group_check=True,
                )
                add_dep_helper(g2.ins, last_mm.ins, sync=False,
                               reason="warm gate 2")
                last_mm = g2
                junk(20)
            fstub = FAke[:, 10, b1 - 1 : b1, 15:16]
            op = nc.tensor.matmul(
                pjunk[0:1, 120 + si : 121 + si], fstub, fstub,
                start=True, stop=True, skip_group_check=True,
            )
            add_dep_helper(op.ins, last_mm.ins, sync=False,
                           reason="opener after prev MMs")
            opener = op
            last_mm = op
            for tcc in (2 * si, 2 * si + 1):
                for b in range(BPC):
                    ps = pstiles[b]
                    for oc in range(8):
                        blk = (tcc * BPC + b) * 8 + oc
                        mm = nc.tensor.matmul(
                            ps[:], FAm[:, blk, 0], FAm[:, blk, 1],
                            start=(tcc == 0 and oc == 0),
                            stop=(tcc == TC - 1 and oc == 7),
                            skip_group_check=True,
                        )
                        add_dep_helper(mm.ins, opener.ins, sync=False,
                                       reason="PE wait-slot opener")
                        last_mm = mm

        # PSUM -> SBUF on ACT; one output DMA per contiguous span so the
        # output transfer overlaps the burst tail instead of serializing
        # 328 KB after the last copy.  probe+psu ride with b0's DMA to stay
        # within the 8 HW queues (4 in + 4 out).
        _pin("act", nc.scalar.copy(
            out=csb[:, NPROBE : NPROBE + 128], in_=psu[:]))
        PS0 = NPROBE + 128
        for b in range(BPC):
            _pin("act", nc.scalar.copy(
                out=csb[:, PS0 + b * 128 : PS0 + (b + 1) * 128],
                in_=pstiles[b][:]))
            lo = 0 if b == 0 else PS0 + b * 128
            hi = PS0 + (b + 1) * 128
            nc.sync.dma_start(out=out_dram[:, lo:hi], in_=csb[:, lo:hi])


_CACHE = {}


def _build():
    if "nc" in _CACHE:
        return _CACHE["nc"]
    # Bass.__init__ gpsimd-memsets four default const tensors and then
    # runs an all-engine barrier; the gpsimd engine takes ~2us just to
    # start its instruction stream, so that barrier delays the first DMA
    # kicks to ~7us.  Stub both out: the only init-const this kernel uses
    # (f32 0.0 activation bias) is remapped to a pr column in _body, and
    # cross-engine ordering is provided by Tile's semaphores.
    _orig_memset = bass.BassSharedVectorInterface.memset
    _orig_barrier = bass.Bass.all_engine_barrier
    bass.BassSharedVectorInterface.memset = lambda self, ap, v: None
    bass.Bass.all_engine_barrier = lambda self, **kw: None
    try:
        nc = bass.Bass("TRN2", debug=False)
    finally:
        bass.BassSharedVectorInterface.memset = _orig_memset
        bass.Bass.all_engine_barrier = _orig_barrier
    # Skip the kernel-tail DGE-queue drain (~3-4us); all DMAs are already
    # completion-waited by the split drains and no dynamic DMA state is
    # used.  Second-execution correctness is validated by the harness.
    type(nc.gpsimd).dma_reset = lambda self, semaphore_range=None: None
    xu_in = nc.dram_tensor("xu", [128, 2 * COLS], F16, kind="ExternalInput")
    pr_in = nc.dram_tensor("pr", [128, NPR], F32, kind="ExternalInput")
    out_d = nc.dram_tensor(
        "out", [128, NPROBE + 128 + BPC * 128], F32, kind="ExternalOutput"
    )
    with TileContext(nc) as t:
        _body(nc, t, xu_in.ap(), pr_in.ap(), out_d.ap())
    _CACHE["nc"] = nc
    return nc


def _shard_host(a):
    """[T, B, N, D] -> per-core [128, COLS] fp16, col = tc*512+b*128+n*2+d."""
    out = []
    for c in range(N_CORES):
        s = a[:, c * BPC : (c + 1) * BPC]  # [512, 4, 64, 2]
        s = s.reshape(TC, 128, BPC, N_AGENTS, D)  # (tc, p, b, n, d)
        s = np.ascontiguousarray(np.transpose(s, (1, 0, 2, 3, 4)))
        out.append(s.reshape(128, COLS).astype(np.float16))
    return out


def _probe_args():
    xp = (np.arange(128, dtype=np.float64) + 0.5) / 128.0
    ks = np.arange(NPROBE, dtype=np.float64)
    a = np.empty((128, NPR), dtype=np.float32)
    a[:, :NPROBE] = (ks[None, :] * np.pi * xp[:, None] - np.pi / 2)
    a[:, NPROBE] = -np.pi / 2
    a[:, NPROBE + 1] = -1.0
    a[:, NPROBE + 2] = 0.0
    return a


def _make_inmaps(x, u):
    xs = _shard_host(np.asarray(x, dtype=np.float32))
    us = _shard_host(np.asarray(u, dtype=np.float32))
    prb = _probe_args()
    return [
        {
            "xu": np.ascontiguousarray(np.concatenate([xs[c], us[c]], axis=1)),
            "pr": prb,
        }
        for c in range(N_CORES)
    ]


def kernel(x, u, **_):
    nc = _build()
    in_maps = _make_inmaps(x, u)
    res = bass_utils.run_bass_kernel_spmd(nc, in_maps, core_ids=list(range(N_CORES)))
    return _finish_host(res.results)


_LAST_PROBE = None


def _finish_host(outs):
    """Host reduction/unmixing in float64 -> scalar loss."""
    global _LAST_PROBE
    Cp = np.zeros((B, K_MAX, K_MAX), dtype=np.float64)
    u2 = 0.0
    PS0 = NPROBE + 128
    for c in range(N_CORES):
        o = outs[c]["out"].astype(np.float64)  # [128, 656]
        ublk = o[:, NPROBE:PS0]
        u2 += float(np.trace(ublk))
        for b in range(BPC):
            blk = o[:, PS0 + b * 128 : PS0 + (b + 1) * 128]
            acc = np.zeros((K_MAX, K_MAX))
            for nl in range(8):
                acc += blk[nl::8, nl::8]
            Cp[c * BPC + b] = acc
    o0 = outs[0]["out"].astype(np.float64)
    dev = o0[:, :NPROBE]
    tru = np.sin(_probe_args()[:, :NPROBE].astype(np.float64))
    _LAST_PROBE = np.max(np.abs(dev - tru), axis=0)

    Ct = np.einsum("ik,bkl,jl->bij", _AINV, Cp, _AINV)
    c = Ct / (_NORM[None] * (N_AGENTS * T))
    loss = np.mean((c - _COEFFS[None]) ** 2)
    loss = loss + REG * u2 / (2.0 * N_AGENTS * T * B)
    return np.array(loss, dtype=np.float32)


if __name__ == "__main__":
    rng = np.random.default_rng(0)
    x = rng.random((T, B, N_AGENTS, D), dtype=np.float32)
    u = rng.standard_normal((T, B, N_AGENTS, D)).astype(np.float32)
    print(kernel(x=x, u=u))
    print("probe err per k:", _LAST_PROBE)
